# revision 1
# baseline (speedup 1.0000x reference)
"""GNN linear-attention kernel for Trainium2 (8 NeuronCores).

Sharding: data-parallel over batch B=8 -- one graph (N=2048 nodes) per
NeuronCore; parameters replicated. Inputs are full (unsharded) numpy
arrays; output is the full (B, N, O) float32 array.
"""

import numpy as np

B, N, D, O = 8, 2048, 128, 128

_compiled = {}


def _get_fn():
    import jax
    import jax.numpy as jnp

    if "fn" in _compiled:
        return _compiled["fn"]

    def f(x_b, A_u8, W_qk, b_qk, W_l, b_l, W_r, W_d, b_d):
        d = x_b.shape[-1]
        A_b = A_u8.astype(jnp.float32)
        deg = jnp.sum(A_b, axis=-1, keepdims=True)
        gate = jax.nn.sigmoid(deg @ W_d + b_d)
        xg = x_b * gate
        QK = jax.nn.sigmoid(xg @ W_qk + b_qk)
        scores = (QK @ QK.T) / jnp.sqrt(jnp.float32(d))
        scores = scores * A_b
        attn = scores / (jnp.sum(scores, axis=-1, keepdims=True) + 1e-6)
        agg = attn @ xg
        out = agg @ W_l + b_l + xg @ W_r
        nrm = jnp.linalg.norm(out, axis=-1, keepdims=True)
        return out / jnp.maximum(nrm, 1e-12)

    _compiled["fn"] = jax.jit(f)
    _compiled["pfn"] = jax.pmap(
        f, in_axes=(0, 0, None, None, None, None, None, None, None)
    )
    return _compiled["fn"]


def _run_loop(x, A, weights):
    import jax

    fn = _get_fn()
    devs = jax.devices()
    ndev = min(len(devs), x.shape[0])
    futs = []
    for b in range(x.shape[0]):
        dev = devs[b % ndev]
        args = [jax.device_put(np.asarray(t), dev) for t in (x[b], A[b]) + weights]
        futs.append(fn(*args))
    return np.stack([np.asarray(f_) for f_ in futs], axis=0)


def kernel(x, A, W_qk, b_qk, W_l, b_l, W_r, W_d, b_d):
    weights = (W_qk, b_qk, W_l, b_l, W_r, W_d, b_d)
    _get_fn()
    # A is a 0/1 adjacency matrix: ship it as uint8 (lossless, 4x less
    # transfer) and cast back to f32 on-device.
    A_u8 = A.astype(np.uint8)
    try:
        # One parallel dispatch: one graph per NeuronCore.
        out = np.asarray(_compiled["pfn"](x, A_u8, *weights))
    except Exception:
        out = _run_loop(x, A_u8, weights)
    return out.astype(np.float32)



# revision 10
# speedup vs baseline: 2.0759x; 2.0759x over previous
"""GNN linear-attention kernel for Trainium2 (8 NeuronCores, Bass/Tile).

Sharding: data-parallel over batch B=8 -- one graph (N=2048 nodes) per
NeuronCore; parameters replicated. Host packs each graph's inputs into a
single uint8 blob (x^T and weights as bf16, adjacency bitpacked 8:1, degrees
precomputed) so the whole batch ships as ONE sharded host->device transfer;
the Bass kernel unpacks the adjacency bits on-device, runs the full
gate/QK/masked-attention/aggregate/normalize pipeline per core, and returns
bf16 outputs (one d2h transfer), cast to f32 on host.
"""
from contextlib import ExitStack
from concurrent.futures import ThreadPoolExecutor
import math

import numpy as np
import ml_dtypes

B, N, D, O = 8, 2048, 128, 128
P = 128
NPBF16 = ml_dtypes.bfloat16

_cache = {}


# ---------------------------------------------------------------- blob layout
def _blob_layout(n=N, d=D, o=O):
    j = n // 8
    xw_elems = d * (n + d + 2 * o)
    off_xw = 0
    off_pk = off_xw + xw_elems * 2
    off_auxc = off_pk + n * j
    off_auxr = off_auxc + d * 3 * 4
    size = off_auxr + (n + o) * 4
    return dict(J=j, off_xw=off_xw, off_pk=off_pk, off_auxc=off_auxc,
                off_auxr=off_auxr, size=size)


def _pack_core(x_b, A_b, W_qk, b_qk, W_l, b_l, W_r, W_d, b_d, out):
    """Pack one graph into a preallocated uint8 blob row."""
    n, d, o = N, D, O
    j = n // 8
    xw = np.empty((d, n + d + 2 * o), dtype=NPBF16)
    xw[:, 0:n] = x_b.T
    xw[:, n:n + d] = W_qk
    xw[:, n + d:n + d + o] = W_l
    xw[:, n + d + o:] = W_r
    Au8 = A_b.astype(np.uint8)
    pk = np.packbits(Au8.reshape(n, 8, j), axis=1, bitorder="little")
    auxc = np.empty((d, 3), dtype=np.float32)
    auxc[:, 0] = W_d[0]
    auxc[:, 1] = b_d
    auxc[:, 2] = b_qk
    auxr = np.empty(n + o, dtype=np.float32)
    auxr[0:n] = A_b.sum(axis=1, dtype=np.float32)
    auxr[n:] = b_l
    ofs = 0
    for arr in (xw, pk, auxc, auxr):
        bts = arr.view(np.uint8).reshape(-1)
        out[ofs:ofs + bts.size] = bts
        ofs += bts.size
    assert ofs == out.size


# ---------------------------------------------------------------- bass kernel
def _build_nc():
    import concourse.tile as tile
    from concourse import bacc, mybir, masks

    F32 = mybir.dt.float32
    BF16 = mybir.dt.bfloat16
    U8 = mybir.dt.uint8

    lay = _blob_layout()
    J = lay["J"]
    T = N // P
    EPS_RS = 1e-6 * math.sqrt(D)

    nc = bacc.Bacc("TRN2", target_bir_lowering=False, debug=False)
    blob = nc.declare_dram_parameter("blob", [1, lay["size"]], U8, isOutput=False)
    out_d = nc.declare_dram_parameter("out", [N, O], BF16, isOutput=True)

    bl = blob.ap()
    xw_v = bl[:, lay["off_xw"]:lay["off_pk"]].bitcast(BF16) \
        .rearrange("1 (p f) -> p f", p=D)
    pk_v = bl[:, lay["off_pk"]:lay["off_auxc"]] \
        .rearrange("1 (t p j) -> p t j", p=P, j=J)
    auxc_v = bl[:, lay["off_auxc"]:lay["off_auxr"]].bitcast(F32) \
        .rearrange("1 (p f) -> p f", p=D)
    auxr_v = bl[:, lay["off_auxr"]:lay["size"]].bitcast(F32)

    W = N + D + 2 * O

    with tile.TileContext(nc) as tc, ExitStack() as ctx:
        cpool = ctx.enter_context(tc.tile_pool(name="const", bufs=1))
        wpool = ctx.enter_context(tc.tile_pool(name="work", bufs=3))
        spool = ctx.enter_context(tc.tile_pool(name="small", bufs=3))
        ps_s = ctx.enter_context(tc.tile_pool(name="ps_s", bufs=2, space="PSUM"))
        ps_tr = ctx.enter_context(tc.tile_pool(name="ps_tr", bufs=2, space="PSUM"))
        ps_agg = ctx.enter_context(tc.tile_pool(name="ps_agg", bufs=2, space="PSUM"))
        ps_big = ctx.enter_context(tc.tile_pool(name="ps_big", bufs=2, space="PSUM"))

        xw = cpool.tile([D, W], BF16)
        nc.sync.dma_start(xw[:], xw_v)
        pk = cpool.tile([P, T, J], U8)
        nc.sync.dma_start(pk[:], pk_v)
        auxc = cpool.tile([D, 3], F32)
        nc.sync.dma_start(auxc[:], auxc_v)
        auxr_sb = cpool.tile([1, N + O], F32)
        nc.sync.dma_start(auxr_sb[:], auxr_v)
        auxr_bf = cpool.tile([1, N + O], BF16)
        nc.vector.tensor_copy(auxr_bf[:], auxr_sb[:])
        ones_bf = cpool.tile([1, P], BF16)
        nc.vector.memset(ones_bf[:], 1.0)
        ident = cpool.tile([P, P], BF16)
        masks.make_identity(nc, ident[:])

        xt = xw[:, 0:N]
        wqk = xw[:, N:N + D]
        wl = xw[:, N + D:N + D + O]
        wr = xw[:, N + D + O:W]

        # gate/xg in transposed (D, N) layout; deg broadcast across
        # partitions via a K=1 matmul with a ones column (deg is integer
        # valued and small, so bf16 is exact)
        gateT = cpool.tile([D, N], BF16)
        GC = 512
        for c in range(N // GC):
            psg = ps_big.tile([P, GC], F32, tag="big")
            nc.tensor.matmul(psg[:], ones_bf[:],
                             auxr_bf[:, c * GC:(c + 1) * GC],
                             start=True, stop=True)
            nc.scalar.activation(gateT[:, c * GC:(c + 1) * GC], psg[:],
                                 mybir.ActivationFunctionType.Sigmoid,
                                 bias=auxc[:, 1:2], scale=auxc[:, 0:1])
        xgT = cpool.tile([D, N], BF16)
        nc.vector.tensor_tensor(out=xgT[:], in0=xt, in1=gateT[:],
                                op=mybir.AluOpType.mult)

        # QK^T = sigmoid(W_qk^T @ xgT + b_qk)
        QKT = cpool.tile([D, N], BF16)
        QC = 512
        for c in range(N // QC):
            psq = ps_big.tile([P, QC], F32, tag="big")
            nc.tensor.matmul(psq[:], wqk, xgT[:, c * QC:(c + 1) * QC],
                             start=True, stop=True)
            nc.scalar.activation(QKT[:, c * QC:(c + 1) * QC], psq[:],
                                 mybir.ActivationFunctionType.Sigmoid,
                                 bias=auxc[:, 2:3])

        # xg natural layout (m on partitions) via PE transpose
        xgN = cpool.tile([P, T, D], BF16)
        for mt in range(T):
            pst = ps_tr.tile([P, P], BF16, tag="tr")
            nc.tensor.transpose(pst[:], xgT[:, mt * P:(mt + 1) * P], ident[:])
            nc.vector.tensor_copy(xgN[:, mt, :], pst[:])

        for nb in range(T):
            psa = ps_agg.tile([P, D], F32, tag="agg")
            rs_parts = spool.tile([P, T], F32, tag="rsp")
            n0 = nb * P
            for mc in range(T):
                pss = ps_s.tile([P, P], F32, tag="s")
                nc.tensor.matmul(pss[:], QKT[:, n0:n0 + P],
                                 QKT[:, mc * P:(mc + 1) * P],
                                 start=True, stop=True)
                mask_u8 = wpool.tile([P, P], U8, tag="mask_u8")
                m0 = mc * P
                g0, g1 = m0 // J, (m0 + P - 1) // J
                for g in range(g0, g1 + 1):
                    lo, hi = max(J * g, m0), min(J * g + J, m0 + P)
                    nc.vector.tensor_scalar(
                        out=mask_u8[:, lo - m0:hi - m0],
                        in0=pk[:, nb, lo - J * g:hi - J * g],
                        scalar1=g, scalar2=1,
                        op0=mybir.AluOpType.logical_shift_right,
                        op1=mybir.AluOpType.bitwise_and)
                mask = wpool.tile([P, P], BF16, tag="mask")
                nc.vector.tensor_copy(mask[:], mask_u8[:])
                masked = wpool.tile([P, P], BF16, tag="masked")
                nc.vector.tensor_tensor(out=masked[:], in0=pss[:], in1=mask[:],
                                        op=mybir.AluOpType.mult)
                nc.vector.tensor_reduce(out=rs_parts[:, mc:mc + 1], in_=masked[:],
                                        axis=mybir.AxisListType.X,
                                        op=mybir.AluOpType.add)
                pst = ps_tr.tile([P, P], BF16, tag="tr")
                nc.tensor.transpose(pst[:], masked[:], ident[:])
                maskedT = wpool.tile([P, P], BF16, tag="maskedT")
                nc.vector.tensor_copy(maskedT[:], pst[:])
                nc.tensor.matmul(psa[:], maskedT[:], xgN[:, mc, :],
                                 start=(mc == 0), stop=(mc == T - 1))

            rs = spool.tile([P, 1], F32, tag="rs")
            nc.vector.tensor_reduce(out=rs[:], in_=rs_parts[:],
                                    axis=mybir.AxisListType.X,
                                    op=mybir.AluOpType.add)
            rcp = spool.tile([P, 1], F32, tag="rcp")
            nc.vector.tensor_scalar_add(rs[:], rs[:], EPS_RS)
            nc.vector.reciprocal(rcp[:], rs[:])
            agg_sb = spool.tile([P, D], BF16, tag="aggsb")
            nc.vector.tensor_scalar(out=agg_sb[:], in0=psa[:], scalar1=rcp[:],
                                    scalar2=None, op0=mybir.AluOpType.mult)
            pst2 = ps_tr.tile([P, P], BF16, tag="tr")
            nc.tensor.transpose(pst2[:], agg_sb[:], ident[:])
            aggT = spool.tile([P, D], BF16, tag="aggT")
            nc.vector.tensor_copy(aggT[:], pst2[:])

            pso = ps_big.tile([P, O], F32, tag="big")
            nc.tensor.matmul(pso[:], aggT[:], wl, start=True, stop=False)
            nc.tensor.matmul(pso[:], xgT[:, n0:n0 + P], wr, start=False, stop=False)
            nc.tensor.matmul(pso[:], ones_bf[:], auxr_bf[:, N:N + O],
                             start=False, stop=True)

            t = spool.tile([P, O], F32, tag="t")
            nc.vector.tensor_copy(t[:], pso[:])
            sq = spool.tile([P, O], F32, tag="sq")
            ss = spool.tile([P, 1], F32, tag="ss")
            nc.scalar.activation(sq[:], t[:], mybir.ActivationFunctionType.Square,
                                 accum_out=ss[:])
            ssi = spool.tile([P, 1], F32, tag="ssi")
            nc.vector.reciprocal(ssi[:], ss[:])
            rn = spool.tile([P, 1], F32, tag="rn")
            nc.scalar.activation(rn[:], ssi[:], mybir.ActivationFunctionType.Sqrt)
            nc.vector.tensor_scalar_min(rn[:], rn[:], 1e12)
            outb = spool.tile([P, O], BF16, tag="outb")
            nc.vector.tensor_scalar(out=outb[:], in0=t[:], scalar1=rn[:],
                                    scalar2=None, op0=mybir.AluOpType.mult)
            nc.sync.dma_start(out_d[n0:n0 + P, :], outb[:])

    nc.finalize()
    return nc


# ---------------------------------------------------------------- jax runner
def _get_rt():
    if "rt" in _cache:
        return _cache["rt"]
    import jax
    import jax.numpy as jnp
    from jax.experimental.shard_map import shard_map
    from jax.sharding import Mesh, PartitionSpec, NamedSharding
    from concourse import bass2jax, mybir

    nc = _build_nc()
    bass2jax.install_neuronx_cc_hook()

    partition_name = (nc.partition_id_tensor.name
                      if nc.partition_id_tensor else None)
    in_names, out_names, out_avals = [], [], []
    for alloc in nc.m.functions[0].allocations:
        if not isinstance(alloc, mybir.MemoryLocationSet):
            continue
        name = alloc.memorylocations[0].name
        if alloc.kind == "ExternalInput":
            if name != partition_name:
                in_names.append(name)
        elif alloc.kind == "ExternalOutput":
            out_names.append(name)
            out_avals.append(jax.core.ShapedArray(
                tuple(alloc.tensor_shape), mybir.dt.np(alloc.dtype)))
    assert in_names == ["blob"] and out_names == ["out"], (in_names, out_names)
    bind_names = in_names + out_names
    if partition_name is not None:
        bind_names = bind_names + [partition_name]

    def _body(*args):
        operands = list(args)
        if partition_name is not None:
            operands.append(bass2jax.partition_id_tensor())
        outs = bass2jax._bass_exec_p.bind(
            *operands,
            out_avals=tuple(out_avals),
            in_names=tuple(bind_names),
            out_names=tuple(out_names),
            lowering_input_output_aliases=(),
            sim_require_finite=True,
            sim_require_nnan=True,
            nc=nc,
        )
        return tuple(outs)

    devices = jax.devices()[:B]
    mesh = Mesh(np.asarray(devices), ("core",))
    spec = PartitionSpec("core")
    sharded = jax.jit(
        shard_map(_body, mesh=mesh, in_specs=(spec, spec),
                  out_specs=(spec,), check_rep=False),
        donate_argnums=(1,), keep_unused=True)
    zeros_fn = jax.jit(
        lambda: jnp.zeros((B * N, O), jnp.bfloat16),
        out_shardings=NamedSharding(mesh, spec))
    in_sharding = NamedSharding(mesh, spec)

    rt = dict(sharded=sharded, zeros_fn=zeros_fn, in_sharding=in_sharding,
              jax=jax)
    _cache["rt"] = rt
    return rt


def kernel(x, A, W_qk, b_qk, W_l, b_l, W_r, W_d, b_d):
    rt = _get_rt()
    jax = rt["jax"]

    zeros = rt["zeros_fn"]()  # device-resident, donated to the NEFF output

    lay = _blob_layout()
    blob = np.empty((B, lay["size"]), dtype=np.uint8)
    args = (W_qk, b_qk, W_l, b_l, W_r, W_d, b_d)
    with ThreadPoolExecutor(B) as ex:
        list(ex.map(lambda b: _pack_core(x[b], A[b], *args, out=blob[b]),
                    range(B)))

    dev_blob = jax.device_put(blob, rt["in_sharding"])
    (out_g,) = rt["sharded"](dev_blob, zeros)
    res = np.asarray(out_g)
    return res.reshape(B, N, O).astype(np.float32)
